# revision 21
# baseline (speedup 1.0000x reference)
"""Two-layer GraphConv (DGL norm='both') on 8 Trainium2 NeuronCores.

Strategy (dst-sharded graph parallel, v2 "flat streams"):
  - Nodes split into 8 contiguous shards of 12500; core c owns dst-shard c and
    the ~200k edges whose dst lands in it.
  - Per layer: each core computes hW = (h * norm_src) @ W for its own 12500
    nodes (bf16), then per-fragment AllGathers assemble the full 100k x 128
    table in every core's DRAM (4 fragments of 25000 rows, rank-major, which
    double as the int16 gather windows).
  - Gather: per window w (= SWDGE queue w) the edges of all 98 dst tiles are
    packed into ONE flat slot stream (per-(tile,window) capacity = max over
    cores, no chunk-granularity padding).  dma_gather calls of CALL_CHUNKS*128
    slots each stream down queue w independently; pad slots point at row 0
    (valid data, killed by one-hot zeros).  Constant num_idxs - no count
    registers.  Fewer, larger calls amortize the ~0.85us/call fixed cost of
    the Pool sequencer that dominated v1.
  - Segment-sum over dst on the TensorEngine: chunk j of stream w is matmul'd
    (one-hot lhsT built on VectorE from a per-slot dst-local table) into the
    PSUM accumulator of each dst tile it covers; chunks at tile boundaries
    get one matmul per covered tile.  Tile-major matmul order, PSUM
    accumulation across all 4 windows of a tile, then a fused epilogue
    (relu(agg*scale) folding norm_dst and the next layer's norm_src).

One SPMD program on all cores; per-core graph structure lives in the input
data (idx stream + dst-local one-hot columns).
"""

import os
import numpy as np
import ml_dtypes

N_NODES = 100000
N_EDGES = 1600000
D = 128
NC = 8
P = 128
SHARD = N_NODES // NC            # 12500
TILES = (SHARD + P - 1) // P     # 98 dst tiles/core (last tile 84 valid rows)
SHARD_PAD = TILES * P            # 12544
NW = 4
FR = SHARD // NW                 # 3125 local rows per fragment
WROWS = NC * FR                  # 25000 rows per gather window

# 8 chunks (1024 idxs, 65 descriptors/engine) is the largest call that the
# SWDGE descriptor ring accepts on HW: 10/14/15/16-chunk calls all hang in
# the ring await, and smaller calls pay more of the ~1us/call serial cost.
CALL_CHUNKS = int(os.environ.get("CCAS_CALLCH", "8"))    # 128-slot chunks/call
CALL_IDX = CALL_CHUNKS * P
NQUEUES = 4
SCRATCH = int(os.environ.get("CCAS_SCRATCH", "16384"))
GBUFS = int(os.environ.get("CCAS_GBUFS", "6"))
OBUFS = int(os.environ.get("CCAS_OBUFS", "3"))
PSAGG = int(os.environ.get("CCAS_PSAGG", "4"))
SKEW = int(os.environ.get("CCAS_SKEW", "0"))   # per-window call-issue head start

BF16 = ml_dtypes.bfloat16

_cache = {}


def _plan(src, dst):
    """Host-side partitioning -> shared schedule + per-core data."""
    deg_out = np.bincount(src, minlength=N_NODES)
    deg_in = np.bincount(dst, minlength=N_NODES)
    norm_src = 1.0 / np.sqrt(np.maximum(deg_out, 1.0))
    norm_dst = 1.0 / np.sqrt(np.maximum(deg_in, 1.0))

    shard_of = dst // SHARD
    src_r = src // SHARD
    src_l = src % SHARD
    win_of = src_l // FR
    frag_row = src_r * FR + src_l % FR

    counts = np.zeros((NC, NW, TILES), np.int64)
    per_core = []
    for c in range(NC):
        m = shard_of == c
        es, ew = frag_row[m], win_of[m]
        dloc = dst[m] - c * SHARD
        tl = dloc // P
        order = np.lexsort((es, tl, ew))
        es, ew, tl, dloc = es[order], ew[order], tl[order], dloc[order]
        np.add.at(counts[c], (ew, tl), 1)
        per_core.append((es, ew, tl, dloc))

    cap = counts.max(axis=0)                       # [NW, TILES]
    # slot offsets of bucket (w, t) inside stream w
    off = np.zeros((NW, TILES + 1), np.int64)
    np.cumsum(cap, axis=1, out=off[:, 1:])
    stream_len = off[:, -1].copy()                 # [NW]
    ncalls = [int(-(-stream_len[w] // CALL_IDX)) for w in range(NW)]
    stream_pad = [ncalls[w] * CALL_IDX for w in range(NW)]

    # matmul units (t, w, chunk).  Matmul schedule is tile-major; the dstl
    # column layout is call-major (w, chunk, t) so each call's one-hot build
    # reads a contiguous column range.
    units = []          # list of (t, w, chunk), tile-major
    for t in range(TILES):
        for w in range(NW):
            lo, hi = int(off[w, t]), int(off[w, t + 1])
            if hi == lo:
                continue
            for j in range(lo // P, (hi - 1) // P + 1):
                units.append((t, w, j))
    n_units = len(units)
    order_cm = sorted(range(n_units), key=lambda u: (units[u][1],
                                                     units[u][2],
                                                     units[u][0]))
    dstl_col = [0] * n_units     # unit -> dstl column (call-major position)
    for pos, u in enumerate(order_cm):
        dstl_col[u] = pos

    # per-core data: idx streams + dstl columns
    idx_cols = sum(stream_pad) // 16
    idx_all = np.zeros((NC, 16, idx_cols), np.int16)
    dstl_all = np.full((NC, P, n_units), -1.0, BF16)
    col0 = [0] * NW                                # idx col offset per window
    acc = 0
    for w in range(NW):
        col0[w] = acc
        acc += stream_pad[w] // 16

    for c in range(NC):
        es, ew, tl, dloc = per_core[c]
        for w in range(NW):
            sl = np.zeros(stream_pad[w], np.int16)     # pad slots -> row 0
            dl = np.full(stream_pad[w], -1, np.int16)  # pad slots -> no dst
            m = ew == w
            es_w, tl_w, dl_w = es[m], tl[m], dloc[m]
            # bucket (w, t) of this core occupies [off[w,t], off[w,t]+n_ct)
            n_per_t = np.bincount(tl_w, minlength=TILES)
            starts = off[w, :-1]
            pos = np.repeat(starts, n_per_t) + (
                np.arange(len(tl_w)) - np.repeat(
                    np.cumsum(n_per_t) - n_per_t, n_per_t))
            sl[pos] = es_w.astype(np.int16)
            dl[pos] = (dl_w % P).astype(np.int16)
            idx_all[c, :, col0[w]:col0[w] + stream_pad[w] // 16] = \
                sl.reshape(-1, 16).T
            # dstl columns for this window's units (call-major layout)
            for u in range(len(units)):
                t, uw, j = units[u]
                if uw != w:
                    continue
                seg = dl[j * P:(j + 1) * P]
                ar = np.arange(j * P, (j + 1) * P)
                colv = np.where(
                    (ar >= off[w, t]) & (ar < off[w, t + 1]) & (seg >= 0),
                    seg, -1).astype(BF16)
                dstl_all[c, :, dstl_col[u]] = colv

    def tilemajor(v, c):
        out = np.ones((SHARD_PAD,), np.float32)
        out[:SHARD] = v[c * SHARD:(c + 1) * SHARD]
        return np.ascontiguousarray(out.reshape(TILES, P).T)

    ns_tm = np.stack([tilemajor(norm_src, c) for c in range(NC)])
    nd_tm = np.stack([tilemajor(norm_dst, c) for c in range(NC)])

    plan = dict(units=units, ncalls=ncalls, col0=col0,
                idx_cols=idx_cols, n_units=n_units, dstl_col=dstl_col)
    data = dict(idx_all=idx_all, dstl_all=dstl_all, ns_tm=ns_tm, nd_tm=nd_tm)
    return plan, data


def _build(plan, with_bias):
    import concourse.bass as bass
    import concourse.mybir as mybir
    import concourse.tile as tile
    from concourse import bacc
    from concourse.masks import make_identity

    f32 = mybir.dt.float32
    bf16 = mybir.dt.bfloat16

    units = plan["units"]
    ncalls = plan["ncalls"]
    col0 = plan["col0"]
    idx_cols = plan["idx_cols"]
    n_units = plan["n_units"]
    dstl_col = plan["dstl_col"]

    # per call k of window w: its units, in call-major (dstl column) order
    call_units = {(w, k): [] for w in range(NW) for k in range(ncalls[w])}
    for u in range(n_units):
        t, w, j = units[u]
        call_units[(w, j // CALL_CHUNKS)].append(u)
    for key in call_units:
        call_units[key].sort(key=lambda u: dstl_col[u])
    u_max = max((len(v) for v in call_units.values()), default=1)

    nc = bacc.Bacc("TRN2", target_bir_lowering=False, debug=False,
                   num_devices=NC, num_swdge_queues=NQUEUES,
                   dynamic_dma_scratch_size=SCRATCH)

    x_in = nc.dram_tensor("x_in", [D, SHARD_PAD], bf16, kind="ExternalInput")
    w1_in = nc.dram_tensor("w1_in", [D, D], bf16, kind="ExternalInput")
    w2_in = nc.dram_tensor("w2_in", [D, D], bf16, kind="ExternalInput")
    idx_in = nc.dram_tensor("idx_in", [P, idx_cols], mybir.dt.int16,
                            kind="ExternalInput")
    dstl_in = nc.dram_tensor("dstl_in", [P, n_units], bf16,
                             kind="ExternalInput")
    ns_in = nc.dram_tensor("ns_in", [P, TILES], f32, kind="ExternalInput")
    nd_in = nc.dram_tensor("nd_in", [P, TILES], f32, kind="ExternalInput")
    nds_in = nc.dram_tensor("nds_in", [P, TILES], f32, kind="ExternalInput")
    if with_bias:
        b1_in = nc.dram_tensor("b1_in", [P, D], f32, kind="ExternalInput")
        b2_in = nc.dram_tensor("b2_in", [P, D], f32, kind="ExternalInput")
    y_out = nc.dram_tensor("y_out", [SHARD, D], f32, kind="ExternalOutput")

    ag1_in = nc.dram_tensor("ag1_in", [SHARD, D], bf16, kind="Internal")
    ag2_in = nc.dram_tensor("ag2_in", [SHARD, D], bf16, kind="Internal")
    hw1_frag = [nc.dram_tensor(f"hw1_frag{k}", [WROWS, D], bf16,
                               kind="Internal", addr_space="Shared")
                for k in range(NW)]
    hw2_frag = [nc.dram_tensor(f"hw2_frag{k}", [WROWS, D], bf16,
                               kind="Internal", addr_space="Shared")
                for k in range(NW)]

    RELU = mybir.ActivationFunctionType.Relu
    COPY = mybir.ActivationFunctionType.Copy

    with tile.TileContext(nc) as tc:
        with (
            tc.tile_pool(name="const", bufs=1) as const,
            tc.tile_pool(name="xio", bufs=6) as xio,
            tc.tile_pool(name="g0", bufs=GBUFS) as g0,
            tc.tile_pool(name="g1", bufs=GBUFS) as g1,
            tc.tile_pool(name="g2", bufs=GBUFS) as g2,
            tc.tile_pool(name="g3", bufs=GBUFS) as g3,
            tc.tile_pool(name="o0", bufs=OBUFS) as o0,
            tc.tile_pool(name="o1", bufs=OBUFS) as o1,
            tc.tile_pool(name="o2", bufs=OBUFS) as o2,
            tc.tile_pool(name="o3", bufs=OBUFS) as o3,
            tc.tile_pool(name="ep", bufs=3) as ep,
            tc.tile_pool(name="ps_agg", bufs=PSAGG, space="PSUM") as ps_agg,
            tc.tile_pool(name="ps_tr", bufs=2, space="PSUM") as ps_tr,
            tc.tile_pool(name="ps_mm", bufs=2, space="PSUM") as ps_mm,
        ):
            gpool = [g0, g1, g2, g3]
            opool = [o0, o1, o2, o3]

            # ---- constants ----
            idx_t = const.tile([P, idx_cols], mybir.dt.int16)
            nc.sync.dma_start(out=idx_t[:], in_=idx_in[:])
            dstl_t = const.tile([P, n_units], bf16)
            nc.sync.dma_start(out=dstl_t[:], in_=dstl_in[:])
            ns_t = const.tile([P, TILES], f32)
            nc.sync.dma_start(out=ns_t[:], in_=ns_in[:])
            nd_t = const.tile([P, TILES], f32)
            nc.sync.dma_start(out=nd_t[:], in_=nd_in[:])
            nds_t = const.tile([P, TILES], f32)
            nc.sync.dma_start(out=nds_t[:], in_=nds_in[:])
            w1_t = const.tile([D, D], bf16)
            nc.sync.dma_start(out=w1_t[:], in_=w1_in[:])
            w2_t = const.tile([D, D], bf16)
            nc.sync.dma_start(out=w2_t[:], in_=w2_in[:])
            if with_bias:
                b1_t = const.tile([P, D], f32)
                nc.sync.dma_start(out=b1_t[:], in_=b1_in[:])
                b2_t = const.tile([P, D], f32)
                nc.sync.dma_start(out=b2_t[:], in_=b2_in[:])
            ident = const.tile([P, P], bf16)
            make_identity(nc, ident[:])
            iota_i = const.tile([P, P], mybir.dt.int32)
            nc.gpsimd.iota(iota_i[:], pattern=[[1, P]], base=0,
                           channel_multiplier=0)
            iota_b = const.tile([P, P], bf16)
            nc.vector.tensor_copy(out=iota_b[:], in_=iota_i[:])

            def dense_mm(lhsT, w_t, ag_dst, t):
                mm = ps_mm.tile([P, D], f32, space="PSUM", tag="mm")
                nc.tensor.matmul(mm[:], lhsT=lhsT, rhs=w_t[:], start=True,
                                 stop=True)
                hw_sb = xio.tile([P, D], bf16, tag="hw_sb")
                nc.scalar.activation(hw_sb[:], mm[:], COPY)
                rows = min(SHARD - t * P, P)
                nc.sync.dma_start(out=ag_dst[t * P:t * P + rows, :],
                                  in_=hw_sb[:rows, :])

            # ---- P0: (x * norm_src)^T pre-computed on host -> @W1 -> ag1_in
            for t in range(TILES):
                xT = xio.tile([P, P], bf16, tag="xT")
                nc.sync.dma_start(out=xT[:], in_=x_in[:, t * P:(t + 1) * P])
                dense_mm(xT[:], w1_t, ag1_in, t)

            nidx_reg = nc.gpsimd.to_reg(CALL_IDX)

            # staircase issue order: window w's calls trail window w-1 by
            # SKEW calls so the in-order Pool engine works on window 0 while
            # later windows' AllGathers are still completing.
            issue_order = sorted(
                [(w, k) for w in range(NW) for k in range(ncalls[w])],
                key=lambda wk: (wk[1] + wk[0] * SKEW, wk[0]))

            def agg_phase(frags, layer):
                G = {}
                maxk = max(ncalls)
                for (w, k) in issue_order:
                    g = gpool[w].tile([P, CALL_CHUNKS, D], bf16,
                                      tag=f"G{w}")
                    c0 = col0[w] + k * (CALL_IDX // 16)
                    nc.gpsimd.dma_gather(
                        g[:], frags[w][:],
                        idx_t[:, c0:c0 + CALL_IDX // 16],
                        CALL_IDX, nidx_reg, D, queue_num=w)
                    G[(w, k)] = g
                # one-hot builds: per call
                O = {}
                for k in range(maxk):
                    for w in range(NW):
                        if k >= ncalls[w]:
                            continue
                        us = call_units[(w, k)]
                        if not us:
                            continue
                        nu = len(us)
                        c0 = dstl_col[us[0]]
                        o = opool[w].tile([P, u_max, P], bf16, tag=f"O{w}")
                        nc.vector.tensor_tensor(
                            out=o[:, :nu, :],
                            in0=dstl_t[:, c0:c0 + nu].unsqueeze(2)
                                .to_broadcast([P, nu, P]),
                            in1=iota_b[:].unsqueeze(1)
                                .to_broadcast([P, nu, P]),
                            op=mybir.AluOpType.is_equal)
                        O[(w, k)] = (o, c0)
                # tile-major matmul + epilogue
                ucur = 0
                for t in range(TILES):
                    tus = []
                    while ucur < n_units and units[ucur][0] == t:
                        tus.append(ucur)
                        ucur += 1
                    agg = ps_agg.tile([P, D], f32, space="PSUM", tag="agg")
                    for i, u in enumerate(tus):
                        (_t, w, j) = units[u]
                        k = j // CALL_CHUNKS
                        jj = j % CALL_CHUNKS
                        o, c0 = O[(w, k)]
                        nc.tensor.matmul(
                            agg[:], lhsT=o[:, dstl_col[u] - c0, :],
                            rhs=G[(w, k)][:, jj, :],
                            start=(i == 0), stop=(i == len(tus) - 1))
                    if layer == 1:
                        t2 = ep.tile([P, D], bf16, tag="t2")
                        if with_bias:
                            z = ep.tile([P, D], f32, tag="z")
                            nc.vector.tensor_tensor(
                                out=z[:], in0=agg[:],
                                in1=nd_t[:, t:t + 1].to_broadcast([P, D]),
                                op=mybir.AluOpType.mult)
                            nc.vector.tensor_add(out=z[:], in0=z[:],
                                                 in1=b1_t[:])
                            nc.scalar.activation(t2[:], z[:], RELU,
                                                 scale=ns_t[:, t:t + 1])
                        else:
                            nc.scalar.activation(t2[:], agg[:], RELU,
                                                 scale=nds_t[:, t:t + 1])
                        tp = ps_tr.tile([P, P], bf16, space="PSUM", tag="tr")
                        nc.tensor.transpose(tp[:], t2[:], ident[:])
                        t2T = ep.tile([P, P], bf16, tag="t2T")
                        nc.scalar.activation(t2T[:], tp[:], COPY)
                        dense_mm(t2T[:], w2_t, ag2_in, t)
                    else:
                        y = ep.tile([P, D], f32, tag="y")
                        if with_bias:
                            z = ep.tile([P, D], f32, tag="z")
                            nc.vector.tensor_tensor(
                                out=z[:], in0=agg[:],
                                in1=nd_t[:, t:t + 1].to_broadcast([P, D]),
                                op=mybir.AluOpType.mult)
                            nc.vector.tensor_add(out=z[:], in0=z[:],
                                                 in1=b2_t[:])
                            nc.scalar.activation(y[:], z[:], RELU)
                        else:
                            nc.scalar.activation(y[:], agg[:], RELU,
                                                 scale=nd_t[:, t:t + 1])
                        rows = min(SHARD - t * P, P)
                        nc.sync.dma_start(out=y_out[t * P:t * P + rows, :],
                                          in_=y[:rows, :])

            for k in range(NW):
                nc.gpsimd.collective_compute(
                    "AllGather", mybir.AluOpType.bypass,
                    replica_groups=[list(range(NC))],
                    ins=[ag1_in[k * FR:(k + 1) * FR, :]],
                    outs=[hw1_frag[k][:]])
            agg_phase(hw1_frag, layer=1)
            for k in range(NW):
                nc.gpsimd.collective_compute(
                    "AllGather", mybir.AluOpType.bypass,
                    replica_groups=[list(range(NC))],
                    ins=[ag2_in[k * FR:(k + 1) * FR, :]],
                    outs=[hw2_frag[k][:]])
            agg_phase(hw2_frag, layer=2)

    nc.compile()
    return nc


def kernel(x, W1, b1, W2, b2, src, dst):
    from concourse.bass_utils import run_bass_kernel_spmd

    src = np.asarray(src).astype(np.int64)
    dst = np.asarray(dst).astype(np.int64)
    x = np.asarray(x, dtype=np.float32)
    W1 = np.asarray(W1, dtype=np.float32)
    W2 = np.asarray(W2, dtype=np.float32)
    b1 = np.asarray(b1, dtype=np.float32)
    b2 = np.asarray(b2, dtype=np.float32)

    plan, data = _plan(src, dst)
    with_bias = bool(np.any(b1) or np.any(b2))

    key = (with_bias, CALL_CHUNKS, SCRATCH, GBUFS, OBUFS, PSAGG, SKEW,
           repr(plan["ncalls"]), plan["n_units"], plan["idx_cols"])
    key = hash(key)
    if key not in _cache:
        _cache[key] = _build(plan, with_bias)
    nc = _cache[key]

    norm_src_full = 1.0 / np.sqrt(np.maximum(
        np.bincount(src, minlength=N_NODES), 1.0))
    in_maps = []
    for c in range(NC):
        xp = np.zeros((SHARD_PAD, D), np.float32)
        xp[:SHARD] = (x[c * SHARD:(c + 1) * SHARD]
                      * norm_src_full[c * SHARD:(c + 1) * SHARD, None])
        m = dict(
            x_in=np.ascontiguousarray(xp.T).astype(BF16),
            w1_in=W1.astype(BF16),
            w2_in=W2.astype(BF16),
            idx_in=np.tile(data["idx_all"][c], (8, 1)),
            dstl_in=data["dstl_all"][c],
            ns_in=data["ns_tm"][c],
            nd_in=data["nd_tm"][c],
            nds_in=data["nd_tm"][c] * data["ns_tm"][c],
        )
        if with_bias:
            m["b1_in"] = np.broadcast_to(b1, (P, D)).astype(np.float32).copy()
            m["b2_in"] = np.broadcast_to(b2, (P, D)).astype(np.float32).copy()
        in_maps.append(m)

    prof_dir = os.environ.get("CCAS_PROFILE_DIR")
    if prof_dir:
        import sys, types
        if "antenv.axon_hooks" not in sys.modules:
            import antenv
            mod = types.ModuleType("antenv.axon_hooks")
            mod._hook = None
            mod.set_axon_ntff_profile_hook = lambda h: setattr(mod, "_hook", h)
            mod.get_axon_ntff_profile_hook = lambda: mod._hook
            sys.modules["antenv.axon_hooks"] = mod
            antenv.axon_hooks = mod
            from trn_agent_boot.trn_boot import _ntff_profile_via_ctypes
            mod.set_axon_ntff_profile_hook(
                _ntff_profile_via_ctypes("/opt/axon/libaxon_pjrt.so"))
        from antenv.axon_hooks import get_axon_ntff_profile_hook
        res = run_bass_kernel_spmd(nc, in_maps, core_ids=list(range(NC)))
        hook = get_axon_ntff_profile_hook()
        with hook(prof_dir, list(range(NC))):
            res = run_bass_kernel_spmd(nc, in_maps, core_ids=list(range(NC)))
    else:
        res = run_bass_kernel_spmd(nc, in_maps, core_ids=list(range(NC)))

    return np.concatenate([res.results[c]["y_out"] for c in range(NC)], axis=0)


# revision 22
# speedup vs baseline: 1.0042x; 1.0042x over previous
"""Two-layer GraphConv (DGL norm='both') on 8 Trainium2 NeuronCores.

Strategy (dst-sharded graph parallel, v2 "flat streams"):
  - Nodes split into 8 contiguous shards of 12500; core c owns dst-shard c and
    the ~200k edges whose dst lands in it.
  - Per layer: each core computes hW = (h * norm_src) @ W for its own 12500
    nodes (bf16), then per-fragment AllGathers assemble the full 100k x 128
    table in every core's DRAM (4 fragments of 25000 rows, rank-major, which
    double as the int16 gather windows).
  - Gather: per window w (= SWDGE queue w) the edges of all 98 dst tiles are
    packed into ONE flat slot stream (per-(tile,window) capacity = max over
    cores, no chunk-granularity padding).  dma_gather calls of CALL_CHUNKS*128
    slots each stream down queue w independently; pad slots point at row 0
    (valid data, killed by one-hot zeros).  Constant num_idxs - no count
    registers.  Fewer, larger calls amortize the ~0.85us/call fixed cost of
    the Pool sequencer that dominated v1.
  - Segment-sum over dst on the TensorEngine: chunk j of stream w is matmul'd
    (one-hot lhsT built on VectorE from a per-slot dst-local table) into the
    PSUM accumulator of each dst tile it covers; chunks at tile boundaries
    get one matmul per covered tile.  Tile-major matmul order, PSUM
    accumulation across all 4 windows of a tile, then a fused epilogue
    (relu(agg*scale) folding norm_dst and the next layer's norm_src).

One SPMD program on all cores; per-core graph structure lives in the input
data (idx stream + dst-local one-hot columns).
"""

import os
import numpy as np
import ml_dtypes

N_NODES = 100000
N_EDGES = 1600000
D = 128
NC = 8
P = 128
SHARD = N_NODES // NC            # 12500
TILES = (SHARD + P - 1) // P     # 98 dst tiles/core (last tile 84 valid rows)
SHARD_PAD = TILES * P            # 12544
NW = 4
FR = SHARD // NW                 # 3125 local rows per fragment
WROWS = NC * FR                  # 25000 rows per gather window

# 8 chunks (1024 idxs, 65 descriptors/engine) is the largest call that the
# SWDGE descriptor ring accepts on HW: 10/14/15/16-chunk calls all hang in
# the ring await, and smaller calls pay more of the ~1us/call serial cost.
CALL_CHUNKS = int(os.environ.get("CCAS_CALLCH", "8"))    # 128-slot chunks/call
CALL_IDX = CALL_CHUNKS * P
NQUEUES = 4
SCRATCH = int(os.environ.get("CCAS_SCRATCH", "16384"))
GBUFS = int(os.environ.get("CCAS_GBUFS", "6"))
OBUFS = int(os.environ.get("CCAS_OBUFS", "3"))
PSAGG = int(os.environ.get("CCAS_PSAGG", "4"))
SKEW = int(os.environ.get("CCAS_SKEW", "0"))   # per-window call-issue head start

BF16 = ml_dtypes.bfloat16

_cache = {}


def _plan(src, dst):
    """Host-side partitioning -> shared schedule + per-core data."""
    deg_out = np.bincount(src, minlength=N_NODES)
    deg_in = np.bincount(dst, minlength=N_NODES)
    norm_src = 1.0 / np.sqrt(np.maximum(deg_out, 1.0))
    norm_dst = 1.0 / np.sqrt(np.maximum(deg_in, 1.0))

    shard_of = dst // SHARD
    src_r = src // SHARD
    src_l = src % SHARD
    # table = rank-major full AllGather output: row = src_r*SHARD + src_l;
    # gather window w covers rows [w*WROWS, (w+1)*WROWS)
    win_of = src_r // 2
    frag_row = (src_r % 2) * SHARD + src_l

    counts = np.zeros((NC, NW, TILES), np.int64)
    per_core = []
    for c in range(NC):
        m = shard_of == c
        es, ew = frag_row[m], win_of[m]
        dloc = dst[m] - c * SHARD
        tl = dloc // P
        order = np.lexsort((es, tl, ew))
        es, ew, tl, dloc = es[order], ew[order], tl[order], dloc[order]
        np.add.at(counts[c], (ew, tl), 1)
        per_core.append((es, ew, tl, dloc))

    cap = counts.max(axis=0)                       # [NW, TILES]
    # slot offsets of bucket (w, t) inside stream w
    off = np.zeros((NW, TILES + 1), np.int64)
    np.cumsum(cap, axis=1, out=off[:, 1:])
    stream_len = off[:, -1].copy()                 # [NW]
    ncalls = [int(-(-stream_len[w] // CALL_IDX)) for w in range(NW)]
    stream_pad = [ncalls[w] * CALL_IDX for w in range(NW)]

    # matmul units (t, w, chunk).  Matmul schedule is tile-major; the dstl
    # column layout is call-major (w, chunk, t) so each call's one-hot build
    # reads a contiguous column range.
    units = []          # list of (t, w, chunk), tile-major
    for t in range(TILES):
        for w in range(NW):
            lo, hi = int(off[w, t]), int(off[w, t + 1])
            if hi == lo:
                continue
            for j in range(lo // P, (hi - 1) // P + 1):
                units.append((t, w, j))
    n_units = len(units)
    order_cm = sorted(range(n_units), key=lambda u: (units[u][1],
                                                     units[u][2],
                                                     units[u][0]))
    dstl_col = [0] * n_units     # unit -> dstl column (call-major position)
    for pos, u in enumerate(order_cm):
        dstl_col[u] = pos

    # per-core data: idx streams + dstl columns
    idx_cols = sum(stream_pad) // 16
    idx_all = np.zeros((NC, 16, idx_cols), np.int16)
    dstl_all = np.full((NC, P, n_units), -1.0, BF16)
    col0 = [0] * NW                                # idx col offset per window
    acc = 0
    for w in range(NW):
        col0[w] = acc
        acc += stream_pad[w] // 16

    for c in range(NC):
        es, ew, tl, dloc = per_core[c]
        for w in range(NW):
            sl = np.zeros(stream_pad[w], np.int16)     # pad slots -> row 0
            dl = np.full(stream_pad[w], -1, np.int16)  # pad slots -> no dst
            m = ew == w
            es_w, tl_w, dl_w = es[m], tl[m], dloc[m]
            # bucket (w, t) of this core occupies [off[w,t], off[w,t]+n_ct)
            n_per_t = np.bincount(tl_w, minlength=TILES)
            starts = off[w, :-1]
            pos = np.repeat(starts, n_per_t) + (
                np.arange(len(tl_w)) - np.repeat(
                    np.cumsum(n_per_t) - n_per_t, n_per_t))
            sl[pos] = es_w.astype(np.int16)
            dl[pos] = (dl_w % P).astype(np.int16)
            idx_all[c, :, col0[w]:col0[w] + stream_pad[w] // 16] = \
                sl.reshape(-1, 16).T
            # dstl columns for this window's units (call-major layout)
            for u in range(len(units)):
                t, uw, j = units[u]
                if uw != w:
                    continue
                seg = dl[j * P:(j + 1) * P]
                ar = np.arange(j * P, (j + 1) * P)
                colv = np.where(
                    (ar >= off[w, t]) & (ar < off[w, t + 1]) & (seg >= 0),
                    seg, -1).astype(BF16)
                dstl_all[c, :, dstl_col[u]] = colv

    def tilemajor(v, c):
        out = np.ones((SHARD_PAD,), np.float32)
        out[:SHARD] = v[c * SHARD:(c + 1) * SHARD]
        return np.ascontiguousarray(out.reshape(TILES, P).T)

    ns_tm = np.stack([tilemajor(norm_src, c) for c in range(NC)])
    nd_tm = np.stack([tilemajor(norm_dst, c) for c in range(NC)])

    plan = dict(units=units, ncalls=ncalls, col0=col0,
                idx_cols=idx_cols, n_units=n_units, dstl_col=dstl_col)
    data = dict(idx_all=idx_all, dstl_all=dstl_all, ns_tm=ns_tm, nd_tm=nd_tm)
    return plan, data


def _build(plan, with_bias):
    import concourse.bass as bass
    import concourse.mybir as mybir
    import concourse.tile as tile
    from concourse import bacc
    from concourse.masks import make_identity

    f32 = mybir.dt.float32
    bf16 = mybir.dt.bfloat16

    units = plan["units"]
    ncalls = plan["ncalls"]
    col0 = plan["col0"]
    idx_cols = plan["idx_cols"]
    n_units = plan["n_units"]
    dstl_col = plan["dstl_col"]

    # per call k of window w: its units, in call-major (dstl column) order
    call_units = {(w, k): [] for w in range(NW) for k in range(ncalls[w])}
    for u in range(n_units):
        t, w, j = units[u]
        call_units[(w, j // CALL_CHUNKS)].append(u)
    for key in call_units:
        call_units[key].sort(key=lambda u: dstl_col[u])
    u_max = max((len(v) for v in call_units.values()), default=1)

    nc = bacc.Bacc("TRN2", target_bir_lowering=False, debug=False,
                   num_devices=NC, num_swdge_queues=NQUEUES,
                   dynamic_dma_scratch_size=SCRATCH)

    x_in = nc.dram_tensor("x_in", [D, SHARD_PAD], bf16, kind="ExternalInput")
    w1_in = nc.dram_tensor("w1_in", [D, D], bf16, kind="ExternalInput")
    w2_in = nc.dram_tensor("w2_in", [D, D], bf16, kind="ExternalInput")
    idx_in = nc.dram_tensor("idx_in", [P, idx_cols], mybir.dt.int16,
                            kind="ExternalInput")
    dstl_in = nc.dram_tensor("dstl_in", [P, n_units], bf16,
                             kind="ExternalInput")
    ns_in = nc.dram_tensor("ns_in", [P, TILES], f32, kind="ExternalInput")
    nd_in = nc.dram_tensor("nd_in", [P, TILES], f32, kind="ExternalInput")
    nds_in = nc.dram_tensor("nds_in", [P, TILES], f32, kind="ExternalInput")
    if with_bias:
        b1_in = nc.dram_tensor("b1_in", [P, D], f32, kind="ExternalInput")
        b2_in = nc.dram_tensor("b2_in", [P, D], f32, kind="ExternalInput")
    y_out = nc.dram_tensor("y_out", [SHARD, D], f32, kind="ExternalOutput")

    ag1_in = nc.dram_tensor("ag1_in", [SHARD, D], bf16, kind="Internal")
    ag2_in = nc.dram_tensor("ag2_in", [SHARD, D], bf16, kind="Internal")
    hw1_full = nc.dram_tensor("hw1_full", [NC * SHARD, D], bf16,
                              kind="Internal", addr_space="Shared")
    hw2_full = nc.dram_tensor("hw2_full", [NC * SHARD, D], bf16,
                              kind="Internal", addr_space="Shared")

    RELU = mybir.ActivationFunctionType.Relu
    COPY = mybir.ActivationFunctionType.Copy

    with tile.TileContext(nc) as tc:
        with (
            tc.tile_pool(name="const", bufs=1) as const,
            tc.tile_pool(name="xio", bufs=6) as xio,
            tc.tile_pool(name="g0", bufs=GBUFS) as g0,
            tc.tile_pool(name="g1", bufs=GBUFS) as g1,
            tc.tile_pool(name="g2", bufs=GBUFS) as g2,
            tc.tile_pool(name="g3", bufs=GBUFS) as g3,
            tc.tile_pool(name="o0", bufs=OBUFS) as o0,
            tc.tile_pool(name="o1", bufs=OBUFS) as o1,
            tc.tile_pool(name="o2", bufs=OBUFS) as o2,
            tc.tile_pool(name="o3", bufs=OBUFS) as o3,
            tc.tile_pool(name="ep", bufs=3) as ep,
            tc.tile_pool(name="ps_agg", bufs=PSAGG, space="PSUM") as ps_agg,
            tc.tile_pool(name="ps_tr", bufs=2, space="PSUM") as ps_tr,
            tc.tile_pool(name="ps_mm", bufs=2, space="PSUM") as ps_mm,
        ):
            gpool = [g0, g1, g2, g3]
            opool = [o0, o1, o2, o3]

            # ---- constants ----
            idx_t = const.tile([P, idx_cols], mybir.dt.int16)
            nc.sync.dma_start(out=idx_t[:], in_=idx_in[:])
            dstl_t = const.tile([P, n_units], bf16)
            nc.sync.dma_start(out=dstl_t[:], in_=dstl_in[:])
            ns_t = const.tile([P, TILES], f32)
            nc.sync.dma_start(out=ns_t[:], in_=ns_in[:])
            nd_t = const.tile([P, TILES], f32)
            nc.sync.dma_start(out=nd_t[:], in_=nd_in[:])
            nds_t = const.tile([P, TILES], f32)
            nc.sync.dma_start(out=nds_t[:], in_=nds_in[:])
            w1_t = const.tile([D, D], bf16)
            nc.sync.dma_start(out=w1_t[:], in_=w1_in[:])
            w2_t = const.tile([D, D], bf16)
            nc.sync.dma_start(out=w2_t[:], in_=w2_in[:])
            if with_bias:
                b1_t = const.tile([P, D], f32)
                nc.sync.dma_start(out=b1_t[:], in_=b1_in[:])
                b2_t = const.tile([P, D], f32)
                nc.sync.dma_start(out=b2_t[:], in_=b2_in[:])
            ident = const.tile([P, P], bf16)
            make_identity(nc, ident[:])
            iota_i = const.tile([P, P], mybir.dt.int32)
            nc.gpsimd.iota(iota_i[:], pattern=[[1, P]], base=0,
                           channel_multiplier=0)
            iota_b = const.tile([P, P], bf16)
            nc.vector.tensor_copy(out=iota_b[:], in_=iota_i[:])

            def dense_mm(lhsT, w_t, ag_dst, t):
                mm = ps_mm.tile([P, D], f32, space="PSUM", tag="mm")
                nc.tensor.matmul(mm[:], lhsT=lhsT, rhs=w_t[:], start=True,
                                 stop=True)
                hw_sb = xio.tile([P, D], bf16, tag="hw_sb")
                nc.scalar.activation(hw_sb[:], mm[:], COPY)
                rows = min(SHARD - t * P, P)
                nc.scalar.dma_start(out=ag_dst[t * P:t * P + rows, :],
                                    in_=hw_sb[:rows, :])

            # ---- P0: (x * norm_src)^T pre-computed on host -> @W1 -> ag1_in
            for t0 in range(0, TILES, 4):
                nt = min(4, TILES - t0)
                xT = xio.tile([P, 4 * P], bf16, tag="xT")
                nc.sync.dma_start(out=xT[:, :nt * P],
                                  in_=x_in[:, t0 * P:(t0 + nt) * P])
                for j in range(nt):
                    dense_mm(xT[:, j * P:(j + 1) * P], w1_t, ag1_in, t0 + j)

            nidx_reg = nc.gpsimd.to_reg(CALL_IDX)

            # staircase issue order: window w's calls trail window w-1 by
            # SKEW calls so the in-order Pool engine works on window 0 while
            # later windows' AllGathers are still completing.
            issue_order = sorted(
                [(w, k) for w in range(NW) for k in range(ncalls[w])],
                key=lambda wk: (wk[1] + wk[0] * SKEW, wk[0]))

            def agg_phase(table, layer):
                G = {}
                maxk = max(ncalls)
                for (w, k) in issue_order:
                    g = gpool[w].tile([P, CALL_CHUNKS, D], bf16,
                                      tag=f"G{w}")
                    c0 = col0[w] + k * (CALL_IDX // 16)
                    nc.gpsimd.dma_gather(
                        g[:], table[w * WROWS:(w + 1) * WROWS, :],
                        idx_t[:, c0:c0 + CALL_IDX // 16],
                        CALL_IDX, nidx_reg, D, queue_num=w)
                    G[(w, k)] = g
                # one-hot builds: per call
                O = {}
                for k in range(maxk):
                    for w in range(NW):
                        if k >= ncalls[w]:
                            continue
                        us = call_units[(w, k)]
                        if not us:
                            continue
                        nu = len(us)
                        c0 = dstl_col[us[0]]
                        o = opool[w].tile([P, u_max, P], bf16, tag=f"O{w}")
                        nc.vector.tensor_tensor(
                            out=o[:, :nu, :],
                            in0=dstl_t[:, c0:c0 + nu].unsqueeze(2)
                                .to_broadcast([P, nu, P]),
                            in1=iota_b[:].unsqueeze(1)
                                .to_broadcast([P, nu, P]),
                            op=mybir.AluOpType.is_equal)
                        O[(w, k)] = (o, c0)
                # tile-major matmul + epilogue
                ucur = 0
                for t in range(TILES):
                    tus = []
                    while ucur < n_units and units[ucur][0] == t:
                        tus.append(ucur)
                        ucur += 1
                    agg = ps_agg.tile([P, D], f32, space="PSUM", tag="agg")
                    for i, u in enumerate(tus):
                        (_t, w, j) = units[u]
                        k = j // CALL_CHUNKS
                        jj = j % CALL_CHUNKS
                        o, c0 = O[(w, k)]
                        nc.tensor.matmul(
                            agg[:], lhsT=o[:, dstl_col[u] - c0, :],
                            rhs=G[(w, k)][:, jj, :],
                            start=(i == 0), stop=(i == len(tus) - 1))
                    if layer == 1:
                        t2 = ep.tile([P, D], bf16, tag="t2")
                        if with_bias:
                            z = ep.tile([P, D], f32, tag="z")
                            nc.vector.tensor_tensor(
                                out=z[:], in0=agg[:],
                                in1=nd_t[:, t:t + 1].to_broadcast([P, D]),
                                op=mybir.AluOpType.mult)
                            nc.vector.tensor_add(out=z[:], in0=z[:],
                                                 in1=b1_t[:])
                            nc.scalar.activation(t2[:], z[:], RELU,
                                                 scale=ns_t[:, t:t + 1])
                        else:
                            nc.scalar.activation(t2[:], agg[:], RELU,
                                                 scale=nds_t[:, t:t + 1])
                        tp = ps_tr.tile([P, P], bf16, space="PSUM", tag="tr")
                        nc.tensor.transpose(tp[:], t2[:], ident[:])
                        t2T = ep.tile([P, P], bf16, tag="t2T")
                        nc.scalar.activation(t2T[:], tp[:], COPY)
                        dense_mm(t2T[:], w2_t, ag2_in, t)
                    else:
                        y = ep.tile([P, D], f32, tag="y")
                        if with_bias:
                            z = ep.tile([P, D], f32, tag="z")
                            nc.vector.tensor_tensor(
                                out=z[:], in0=agg[:],
                                in1=nd_t[:, t:t + 1].to_broadcast([P, D]),
                                op=mybir.AluOpType.mult)
                            nc.vector.tensor_add(out=z[:], in0=z[:],
                                                 in1=b2_t[:])
                            nc.scalar.activation(y[:], z[:], RELU)
                        else:
                            nc.scalar.activation(y[:], agg[:], RELU,
                                                 scale=nd_t[:, t:t + 1])
                        rows = min(SHARD - t * P, P)
                        nc.sync.dma_start(out=y_out[t * P:t * P + rows, :],
                                          in_=y[:rows, :])

            nc.gpsimd.collective_compute(
                "AllGather", mybir.AluOpType.bypass,
                replica_groups=[list(range(NC))],
                ins=[ag1_in[:]], outs=[hw1_full[:]])
            agg_phase(hw1_full, layer=1)
            nc.gpsimd.collective_compute(
                "AllGather", mybir.AluOpType.bypass,
                replica_groups=[list(range(NC))],
                ins=[ag2_in[:]], outs=[hw2_full[:]])
            agg_phase(hw2_full, layer=2)

    nc.compile()
    return nc


def kernel(x, W1, b1, W2, b2, src, dst):
    from concourse.bass_utils import run_bass_kernel_spmd

    src = np.asarray(src).astype(np.int64)
    dst = np.asarray(dst).astype(np.int64)
    x = np.asarray(x, dtype=np.float32)
    W1 = np.asarray(W1, dtype=np.float32)
    W2 = np.asarray(W2, dtype=np.float32)
    b1 = np.asarray(b1, dtype=np.float32)
    b2 = np.asarray(b2, dtype=np.float32)

    plan, data = _plan(src, dst)
    with_bias = bool(np.any(b1) or np.any(b2))

    key = (with_bias, CALL_CHUNKS, SCRATCH, GBUFS, OBUFS, PSAGG, SKEW,
           repr(plan["ncalls"]), plan["n_units"], plan["idx_cols"])
    key = hash(key)
    if key not in _cache:
        _cache[key] = _build(plan, with_bias)
    nc = _cache[key]

    norm_src_full = 1.0 / np.sqrt(np.maximum(
        np.bincount(src, minlength=N_NODES), 1.0))
    in_maps = []
    for c in range(NC):
        xp = np.zeros((SHARD_PAD, D), np.float32)
        xp[:SHARD] = (x[c * SHARD:(c + 1) * SHARD]
                      * norm_src_full[c * SHARD:(c + 1) * SHARD, None])
        m = dict(
            x_in=np.ascontiguousarray(xp.T).astype(BF16),
            w1_in=W1.astype(BF16),
            w2_in=W2.astype(BF16),
            idx_in=np.tile(data["idx_all"][c], (8, 1)),
            dstl_in=data["dstl_all"][c],
            ns_in=data["ns_tm"][c],
            nd_in=data["nd_tm"][c],
            nds_in=data["nd_tm"][c] * data["ns_tm"][c],
        )
        if with_bias:
            m["b1_in"] = np.broadcast_to(b1, (P, D)).astype(np.float32).copy()
            m["b2_in"] = np.broadcast_to(b2, (P, D)).astype(np.float32).copy()
        in_maps.append(m)

    prof_dir = os.environ.get("CCAS_PROFILE_DIR")
    if prof_dir:
        import sys, types
        if "antenv.axon_hooks" not in sys.modules:
            import antenv
            mod = types.ModuleType("antenv.axon_hooks")
            mod._hook = None
            mod.set_axon_ntff_profile_hook = lambda h: setattr(mod, "_hook", h)
            mod.get_axon_ntff_profile_hook = lambda: mod._hook
            sys.modules["antenv.axon_hooks"] = mod
            antenv.axon_hooks = mod
            from trn_agent_boot.trn_boot import _ntff_profile_via_ctypes
            mod.set_axon_ntff_profile_hook(
                _ntff_profile_via_ctypes("/opt/axon/libaxon_pjrt.so"))
        from antenv.axon_hooks import get_axon_ntff_profile_hook
        res = run_bass_kernel_spmd(nc, in_maps, core_ids=list(range(NC)))
        hook = get_axon_ntff_profile_hook()
        with hook(prof_dir, list(range(NC))):
            res = run_bass_kernel_spmd(nc, in_maps, core_ids=list(range(NC)))
    else:
        res = run_bass_kernel_spmd(nc, in_maps, core_ids=list(range(NC)))

    return np.concatenate([res.results[c]["y_out"] for c in range(NC)], axis=0)


# revision 25
# speedup vs baseline: 1.0426x; 1.0383x over previous
"""Two-layer GraphConv (DGL norm='both') on 8 Trainium2 NeuronCores.

Strategy (dst-sharded graph parallel, v2 "flat streams"):
  - Nodes split into 8 contiguous shards of 12500; core c owns dst-shard c and
    the ~200k edges whose dst lands in it.
  - Per layer: each core computes hW = (h * norm_src) @ W for its own 12500
    nodes (bf16), then per-fragment AllGathers assemble the full 100k x 128
    table in every core's DRAM (4 fragments of 25000 rows, rank-major, which
    double as the int16 gather windows).
  - Gather: per window w (= SWDGE queue w) the edges of all 98 dst tiles are
    packed into ONE flat slot stream (per-(tile,window) capacity = max over
    cores, no chunk-granularity padding).  dma_gather calls of CALL_CHUNKS*128
    slots each stream down queue w independently; pad slots point at row 0
    (valid data, killed by one-hot zeros).  Constant num_idxs - no count
    registers.  Fewer, larger calls amortize the ~0.85us/call fixed cost of
    the Pool sequencer that dominated v1.
  - Segment-sum over dst on the TensorEngine: chunk j of stream w is matmul'd
    (one-hot lhsT built on VectorE from a per-slot dst-local table) into the
    PSUM accumulator of each dst tile it covers; chunks at tile boundaries
    get one matmul per covered tile.  Tile-major matmul order, PSUM
    accumulation across all 4 windows of a tile, then a fused epilogue
    (relu(agg*scale) folding norm_dst and the next layer's norm_src).

One SPMD program on all cores; per-core graph structure lives in the input
data (idx stream + dst-local one-hot columns).
"""

import os
import numpy as np
import ml_dtypes

N_NODES = 100000
N_EDGES = 1600000
D = 128
NC = 8
P = 128
SHARD = N_NODES // NC            # 12500
TILES = (SHARD + P - 1) // P     # 98 dst tiles/core (last tile 84 valid rows)
SHARD_PAD = TILES * P            # 12544
NW = 4
FR = SHARD // NW                 # 3125 local rows per fragment
RPR = ((TILES + 3) // 4) * 4 * P  # 12800 table rows per rank (group-padded)
WROWS = 2 * RPR                  # 25600 table rows per gather window

# 8 chunks (1024 idxs, 65 descriptors/engine) is the largest call that the
# SWDGE descriptor ring accepts on HW: 10/14/15/16-chunk calls all hang in
# the ring await, and smaller calls pay more of the ~1us/call serial cost.
CALL_CHUNKS = int(os.environ.get("CCAS_CALLCH", "8"))    # 128-slot chunks/call
CALL_IDX = CALL_CHUNKS * P
NQUEUES = 4
SCRATCH = int(os.environ.get("CCAS_SCRATCH", "16384"))
GBUFS = int(os.environ.get("CCAS_GBUFS", "6"))
OBUFS = int(os.environ.get("CCAS_OBUFS", "3"))
PSAGG = int(os.environ.get("CCAS_PSAGG", "4"))
SKEW = int(os.environ.get("CCAS_SKEW", "0"))   # per-window call-issue head start

BF16 = ml_dtypes.bfloat16

_cache = {}


def _plan(src, dst):
    """Host-side partitioning -> shared schedule + per-core data."""
    deg_out = np.bincount(src, minlength=N_NODES)
    deg_in = np.bincount(dst, minlength=N_NODES)
    norm_src = 1.0 / np.sqrt(np.maximum(deg_out, 1.0))
    norm_dst = 1.0 / np.sqrt(np.maximum(deg_in, 1.0))

    shard_of = dst // SHARD
    src_r = src // SHARD
    src_l = src % SHARD
    # table = rank-major full AllGather output with p-major 4-tile groups
    # (epilogue stores batch 4 dst tiles per DMA, partition-major):
    # local row l -> tile t=l//128, p=l%128, group g=t//4, j=t%4,
    # row_in_rank = g*512 + p*nt_g + j where nt_g = tiles in group g.
    t_s = src_l // P
    p_s = src_l % P
    g_s = t_s // 4
    j_s = t_s % 4
    nt_s = np.minimum(4, TILES - 4 * g_s)
    row_in_rank = g_s * (4 * P) + p_s * nt_s + j_s
    win_of = src_r // 2
    frag_row = (src_r % 2) * RPR + row_in_rank

    counts = np.zeros((NC, NW, TILES), np.int64)
    per_core = []
    for c in range(NC):
        m = shard_of == c
        es, ew = frag_row[m], win_of[m]
        dloc = dst[m] - c * SHARD
        tl = dloc // P
        order = np.lexsort((es, tl, ew))
        es, ew, tl, dloc = es[order], ew[order], tl[order], dloc[order]
        np.add.at(counts[c], (ew, tl), 1)
        per_core.append((es, ew, tl, dloc))

    cap = counts.max(axis=0)                       # [NW, TILES]
    # slot offsets of bucket (w, t) inside stream w
    off = np.zeros((NW, TILES + 1), np.int64)
    np.cumsum(cap, axis=1, out=off[:, 1:])
    stream_len = off[:, -1].copy()                 # [NW]
    ncalls = [int(-(-stream_len[w] // CALL_IDX)) for w in range(NW)]
    stream_pad = [ncalls[w] * CALL_IDX for w in range(NW)]

    # matmul units (t, w, chunk).  Matmul schedule is tile-major; the dstl
    # column layout is call-major (w, chunk, t) so each call's one-hot build
    # reads a contiguous column range.
    units = []          # list of (t, w, chunk), tile-major
    for t in range(TILES):
        for w in range(NW):
            lo, hi = int(off[w, t]), int(off[w, t + 1])
            if hi == lo:
                continue
            for j in range(lo // P, (hi - 1) // P + 1):
                units.append((t, w, j))
    n_units = len(units)
    order_cm = sorted(range(n_units), key=lambda u: (units[u][1],
                                                     units[u][2],
                                                     units[u][0]))
    dstl_col = [0] * n_units     # unit -> dstl column (call-major position)
    for pos, u in enumerate(order_cm):
        dstl_col[u] = pos

    # per-core data: idx streams + dstl columns
    idx_cols = sum(stream_pad) // 16
    idx_all = np.zeros((NC, 16, idx_cols), np.int16)
    dstl_all = np.full((NC, P, n_units), -1.0, BF16)
    col0 = [0] * NW                                # idx col offset per window
    acc = 0
    for w in range(NW):
        col0[w] = acc
        acc += stream_pad[w] // 16

    for c in range(NC):
        es, ew, tl, dloc = per_core[c]
        for w in range(NW):
            sl = np.zeros(stream_pad[w], np.int16)     # pad slots -> row 0
            dl = np.full(stream_pad[w], -1, np.int16)  # pad slots -> no dst
            m = ew == w
            es_w, tl_w, dl_w = es[m], tl[m], dloc[m]
            # bucket (w, t) of this core occupies [off[w,t], off[w,t]+n_ct)
            n_per_t = np.bincount(tl_w, minlength=TILES)
            starts = off[w, :-1]
            pos = np.repeat(starts, n_per_t) + (
                np.arange(len(tl_w)) - np.repeat(
                    np.cumsum(n_per_t) - n_per_t, n_per_t))
            sl[pos] = es_w.astype(np.int16)
            dl[pos] = (dl_w % P).astype(np.int16)
            idx_all[c, :, col0[w]:col0[w] + stream_pad[w] // 16] = \
                sl.reshape(-1, 16).T
            # dstl columns for this window's units (call-major layout)
            for u in range(len(units)):
                t, uw, j = units[u]
                if uw != w:
                    continue
                seg = dl[j * P:(j + 1) * P]
                ar = np.arange(j * P, (j + 1) * P)
                colv = np.where(
                    (ar >= off[w, t]) & (ar < off[w, t + 1]) & (seg >= 0),
                    seg, -1).astype(BF16)
                dstl_all[c, :, dstl_col[u]] = colv

    def tilemajor(v, c):
        out = np.ones((SHARD_PAD,), np.float32)
        out[:SHARD] = v[c * SHARD:(c + 1) * SHARD]
        return np.ascontiguousarray(out.reshape(TILES, P).T)

    ns_tm = np.stack([tilemajor(norm_src, c) for c in range(NC)])
    nd_tm = np.stack([tilemajor(norm_dst, c) for c in range(NC)])

    plan = dict(units=units, ncalls=ncalls, col0=col0,
                idx_cols=idx_cols, n_units=n_units, dstl_col=dstl_col)
    data = dict(idx_all=idx_all, dstl_all=dstl_all, ns_tm=ns_tm, nd_tm=nd_tm)
    return plan, data


def _build(plan, with_bias):
    import concourse.bass as bass
    import concourse.mybir as mybir
    import concourse.tile as tile
    from concourse import bacc
    from concourse.masks import make_identity

    f32 = mybir.dt.float32
    bf16 = mybir.dt.bfloat16

    units = plan["units"]
    ncalls = plan["ncalls"]
    col0 = plan["col0"]
    idx_cols = plan["idx_cols"]
    n_units = plan["n_units"]
    dstl_col = plan["dstl_col"]

    # per call k of window w: its units, in call-major (dstl column) order
    call_units = {(w, k): [] for w in range(NW) for k in range(ncalls[w])}
    for u in range(n_units):
        t, w, j = units[u]
        call_units[(w, j // CALL_CHUNKS)].append(u)
    for key in call_units:
        call_units[key].sort(key=lambda u: dstl_col[u])
    u_max = max((len(v) for v in call_units.values()), default=1)

    nc = bacc.Bacc("TRN2", target_bir_lowering=False, debug=False,
                   num_devices=NC, num_swdge_queues=NQUEUES,
                   dynamic_dma_scratch_size=SCRATCH)

    x_in = nc.dram_tensor("x_in", [D, SHARD_PAD], bf16, kind="ExternalInput")
    w1_in = nc.dram_tensor("w1_in", [D, D], bf16, kind="ExternalInput")
    w2_in = nc.dram_tensor("w2_in", [D, D], bf16, kind="ExternalInput")
    idx_in = nc.dram_tensor("idx_in", [P, idx_cols], mybir.dt.int16,
                            kind="ExternalInput")
    dstl_in = nc.dram_tensor("dstl_in", [P, n_units], bf16,
                             kind="ExternalInput")
    ns_in = nc.dram_tensor("ns_in", [P, TILES], f32, kind="ExternalInput")
    nd_in = nc.dram_tensor("nd_in", [P, TILES], f32, kind="ExternalInput")
    nds_in = nc.dram_tensor("nds_in", [P, TILES], f32, kind="ExternalInput")
    if with_bias:
        b1_in = nc.dram_tensor("b1_in", [P, D], f32, kind="ExternalInput")
        b2_in = nc.dram_tensor("b2_in", [P, D], f32, kind="ExternalInput")
    y_out = nc.dram_tensor("y_out", [SHARD, D], f32, kind="ExternalOutput")

    NG = (TILES + 3) // 4        # 4-tile store groups
    ag1_in = nc.dram_tensor("ag1_in", [NG, 4 * P * D], bf16, kind="Internal")
    ag2_in = nc.dram_tensor("ag2_in", [NG, 4 * P * D], bf16, kind="Internal")
    hw1_full = nc.dram_tensor("hw1_full", [NC * RPR, D], bf16,
                              kind="Internal", addr_space="Shared")
    hw2_full = nc.dram_tensor("hw2_full", [NC * RPR, D], bf16,
                              kind="Internal", addr_space="Shared")

    RELU = mybir.ActivationFunctionType.Relu
    COPY = mybir.ActivationFunctionType.Copy

    with tile.TileContext(nc) as tc:
        with (
            tc.tile_pool(name="const", bufs=1) as const,
            tc.tile_pool(name="xio", bufs=6) as xio,
            tc.tile_pool(name="g0", bufs=GBUFS) as g0,
            tc.tile_pool(name="g1", bufs=GBUFS) as g1,
            tc.tile_pool(name="g2", bufs=GBUFS) as g2,
            tc.tile_pool(name="g3", bufs=GBUFS) as g3,
            tc.tile_pool(name="o0", bufs=OBUFS) as o0,
            tc.tile_pool(name="o1", bufs=OBUFS) as o1,
            tc.tile_pool(name="o2", bufs=OBUFS) as o2,
            tc.tile_pool(name="o3", bufs=OBUFS) as o3,
            tc.tile_pool(name="ep", bufs=3) as ep,
            tc.tile_pool(name="ps_agg", bufs=PSAGG, space="PSUM") as ps_agg,
            tc.tile_pool(name="ps_tr", bufs=2, space="PSUM") as ps_tr,
            tc.tile_pool(name="ps_mm", bufs=2, space="PSUM") as ps_mm,
        ):
            gpool = [g0, g1, g2, g3]
            opool = [o0, o1, o2, o3]

            # ---- constants ----
            idx_t = const.tile([P, idx_cols], mybir.dt.int16)
            nc.sync.dma_start(out=idx_t[:], in_=idx_in[:])
            dstl_t = const.tile([P, n_units], bf16)
            nc.sync.dma_start(out=dstl_t[:], in_=dstl_in[:])
            ns_t = const.tile([P, TILES], f32)
            nc.sync.dma_start(out=ns_t[:], in_=ns_in[:])
            nd_t = const.tile([P, TILES], f32)
            nc.sync.dma_start(out=nd_t[:], in_=nd_in[:])
            nds_t = const.tile([P, TILES], f32)
            nc.sync.dma_start(out=nds_t[:], in_=nds_in[:])
            w1_t = const.tile([D, D], bf16)
            nc.sync.dma_start(out=w1_t[:], in_=w1_in[:])
            w2_t = const.tile([D, D], bf16)
            nc.sync.dma_start(out=w2_t[:], in_=w2_in[:])
            if with_bias:
                b1_t = const.tile([P, D], f32)
                nc.sync.dma_start(out=b1_t[:], in_=b1_in[:])
                b2_t = const.tile([P, D], f32)
                nc.sync.dma_start(out=b2_t[:], in_=b2_in[:])
            ident = const.tile([P, P], bf16)
            make_identity(nc, ident[:])
            iota_i = const.tile([P, P], mybir.dt.int32)
            nc.gpsimd.iota(iota_i[:], pattern=[[1, P]], base=0,
                           channel_multiplier=0)
            iota_b = const.tile([P, P], bf16)
            nc.vector.tensor_copy(out=iota_b[:], in_=iota_i[:])

            hwb_state = {}

            def dense_mm(lhsT, w_t, ag_dst, t):
                mm = ps_mm.tile([P, D], f32, space="PSUM", tag="mm")
                nc.tensor.matmul(mm[:], lhsT=lhsT, rhs=w_t[:], start=True,
                                 stop=True)
                g, j = t // 4, t % 4
                nt = min(4, TILES - 4 * g)
                key = id(ag_dst)
                if j == 0:
                    hwb = xio.tile([P, 4 * D], bf16, tag="hwb")
                    hwb_state[key] = hwb
                hwb = hwb_state[key]
                nc.scalar.activation(hwb[:, j * D:(j + 1) * D], mm[:], COPY)
                if j == nt - 1:
                    nc.scalar.dma_start(
                        out=ag_dst[g:g + 1, :nt * P * D],
                        in_=hwb[:, :nt * D])

            # ---- P0: (x * norm_src)^T pre-computed on host -> @W1 -> ag1_in
            for t0 in range(0, TILES, 4):
                nt = min(4, TILES - t0)
                xT = xio.tile([P, 4 * P], bf16, tag="xT")
                nc.sync.dma_start(out=xT[:, :nt * P],
                                  in_=x_in[:, t0 * P:(t0 + nt) * P])
                for j in range(nt):
                    dense_mm(xT[:, j * P:(j + 1) * P], w1_t, ag1_in, t0 + j)

            nidx_reg = nc.gpsimd.to_reg(CALL_IDX)

            # staircase issue order: window w's calls trail window w-1 by
            # SKEW calls so the in-order Pool engine works on window 0 while
            # later windows' AllGathers are still completing.
            issue_order = sorted(
                [(w, k) for w in range(NW) for k in range(ncalls[w])],
                key=lambda wk: (wk[1] + wk[0] * SKEW, wk[0]))

            def agg_phase(table, layer):
                G = {}
                maxk = max(ncalls)
                for (w, k) in issue_order:
                    g = gpool[w].tile([P, CALL_CHUNKS, D], bf16,
                                      tag=f"G{w}")
                    c0 = col0[w] + k * (CALL_IDX // 16)
                    nc.gpsimd.dma_gather(
                        g[:], table[w * WROWS:(w + 1) * WROWS, :],
                        idx_t[:, c0:c0 + CALL_IDX // 16],
                        CALL_IDX, nidx_reg, D, queue_num=w)
                    G[(w, k)] = g
                # one-hot builds: per call
                O = {}
                for k in range(maxk):
                    for w in range(NW):
                        if k >= ncalls[w]:
                            continue
                        us = call_units[(w, k)]
                        if not us:
                            continue
                        nu = len(us)
                        c0 = dstl_col[us[0]]
                        o = opool[w].tile([P, u_max, P], bf16, tag=f"O{w}")
                        nc.vector.tensor_tensor(
                            out=o[:, :nu, :],
                            in0=dstl_t[:, c0:c0 + nu].unsqueeze(2)
                                .to_broadcast([P, nu, P]),
                            in1=iota_b[:].unsqueeze(1)
                                .to_broadcast([P, nu, P]),
                            op=mybir.AluOpType.is_equal)
                        O[(w, k)] = (o, c0)
                # tile-major matmul + epilogue
                ucur = 0
                for t in range(TILES):
                    tus = []
                    while ucur < n_units and units[ucur][0] == t:
                        tus.append(ucur)
                        ucur += 1
                    agg = ps_agg.tile([P, D], f32, space="PSUM", tag="agg")
                    for i, u in enumerate(tus):
                        (_t, w, j) = units[u]
                        k = j // CALL_CHUNKS
                        jj = j % CALL_CHUNKS
                        o, c0 = O[(w, k)]
                        nc.tensor.matmul(
                            agg[:], lhsT=o[:, dstl_col[u] - c0, :],
                            rhs=G[(w, k)][:, jj, :],
                            start=(i == 0), stop=(i == len(tus) - 1))
                    if layer == 1:
                        t2 = ep.tile([P, D], bf16, tag="t2")
                        if with_bias:
                            z = ep.tile([P, D], f32, tag="z")
                            nc.vector.tensor_tensor(
                                out=z[:], in0=agg[:],
                                in1=nd_t[:, t:t + 1].to_broadcast([P, D]),
                                op=mybir.AluOpType.mult)
                            nc.vector.tensor_add(out=z[:], in0=z[:],
                                                 in1=b1_t[:])
                            nc.scalar.activation(t2[:], z[:], RELU,
                                                 scale=ns_t[:, t:t + 1])
                        else:
                            nc.scalar.activation(t2[:], agg[:], RELU,
                                                 scale=nds_t[:, t:t + 1])
                        tp = ps_tr.tile([P, P], bf16, space="PSUM", tag="tr")
                        nc.tensor.transpose(tp[:], t2[:], ident[:])
                        t2T = ep.tile([P, P], bf16, tag="t2T")
                        nc.scalar.activation(t2T[:], tp[:], COPY)
                        dense_mm(t2T[:], w2_t, ag2_in, t)
                    else:
                        y = ep.tile([P, D], f32, tag="y")
                        if with_bias:
                            z = ep.tile([P, D], f32, tag="z")
                            nc.vector.tensor_tensor(
                                out=z[:], in0=agg[:],
                                in1=nd_t[:, t:t + 1].to_broadcast([P, D]),
                                op=mybir.AluOpType.mult)
                            nc.vector.tensor_add(out=z[:], in0=z[:],
                                                 in1=b2_t[:])
                            nc.scalar.activation(y[:], z[:], RELU)
                        else:
                            nc.scalar.activation(y[:], agg[:], RELU,
                                                 scale=nd_t[:, t:t + 1])
                        rows = min(SHARD - t * P, P)
                        nc.sync.dma_start(out=y_out[t * P:t * P + rows, :],
                                          in_=y[:rows, :])

            nc.gpsimd.collective_compute(
                "AllGather", mybir.AluOpType.bypass,
                replica_groups=[list(range(NC))],
                ins=[ag1_in[:]], outs=[hw1_full[:]])
            agg_phase(hw1_full, layer=1)
            nc.gpsimd.collective_compute(
                "AllGather", mybir.AluOpType.bypass,
                replica_groups=[list(range(NC))],
                ins=[ag2_in[:]], outs=[hw2_full[:]])
            agg_phase(hw2_full, layer=2)

    nc.compile()
    return nc


def kernel(x, W1, b1, W2, b2, src, dst):
    from concourse.bass_utils import run_bass_kernel_spmd

    src = np.asarray(src).astype(np.int64)
    dst = np.asarray(dst).astype(np.int64)
    x = np.asarray(x, dtype=np.float32)
    W1 = np.asarray(W1, dtype=np.float32)
    W2 = np.asarray(W2, dtype=np.float32)
    b1 = np.asarray(b1, dtype=np.float32)
    b2 = np.asarray(b2, dtype=np.float32)

    plan, data = _plan(src, dst)
    with_bias = bool(np.any(b1) or np.any(b2))

    key = (with_bias, CALL_CHUNKS, SCRATCH, GBUFS, OBUFS, PSAGG, SKEW,
           repr(plan["ncalls"]), plan["n_units"], plan["idx_cols"])
    key = hash(key)
    if key not in _cache:
        _cache[key] = _build(plan, with_bias)
    nc = _cache[key]

    norm_src_full = 1.0 / np.sqrt(np.maximum(
        np.bincount(src, minlength=N_NODES), 1.0))
    in_maps = []
    for c in range(NC):
        xp = np.zeros((SHARD_PAD, D), np.float32)
        xp[:SHARD] = (x[c * SHARD:(c + 1) * SHARD]
                      * norm_src_full[c * SHARD:(c + 1) * SHARD, None])
        m = dict(
            x_in=np.ascontiguousarray(xp.T).astype(BF16),
            w1_in=W1.astype(BF16),
            w2_in=W2.astype(BF16),
            idx_in=np.tile(data["idx_all"][c], (8, 1)),
            dstl_in=data["dstl_all"][c],
            ns_in=data["ns_tm"][c],
            nd_in=data["nd_tm"][c],
            nds_in=data["nd_tm"][c] * data["ns_tm"][c],
        )
        if with_bias:
            m["b1_in"] = np.broadcast_to(b1, (P, D)).astype(np.float32).copy()
            m["b2_in"] = np.broadcast_to(b2, (P, D)).astype(np.float32).copy()
        in_maps.append(m)

    prof_dir = os.environ.get("CCAS_PROFILE_DIR")
    if prof_dir:
        import sys, types
        if "antenv.axon_hooks" not in sys.modules:
            import antenv
            mod = types.ModuleType("antenv.axon_hooks")
            mod._hook = None
            mod.set_axon_ntff_profile_hook = lambda h: setattr(mod, "_hook", h)
            mod.get_axon_ntff_profile_hook = lambda: mod._hook
            sys.modules["antenv.axon_hooks"] = mod
            antenv.axon_hooks = mod
            from trn_agent_boot.trn_boot import _ntff_profile_via_ctypes
            mod.set_axon_ntff_profile_hook(
                _ntff_profile_via_ctypes("/opt/axon/libaxon_pjrt.so"))
        from antenv.axon_hooks import get_axon_ntff_profile_hook
        res = run_bass_kernel_spmd(nc, in_maps, core_ids=list(range(NC)))
        hook = get_axon_ntff_profile_hook()
        with hook(prof_dir, list(range(NC))):
            res = run_bass_kernel_spmd(nc, in_maps, core_ids=list(range(NC)))
    else:
        res = run_bass_kernel_spmd(nc, in_maps, core_ids=list(range(NC)))

    return np.concatenate([res.results[c]["y_out"] for c in range(NC)], axis=0)
